# revision 58
# baseline (speedup 1.0000x reference)
"""DAGNN (gnn_message_passing) Trainium2 Bass kernel, 8-core SPMD.

Strategy:
  - Nodes padded 100000 -> 100352 (784*128); 8 dst-shards of 12544 rows.
  - Symmetric norm factored into per-node scales:
        feat_{k+1} = Di A Do feat_k,  Di=diag(dinv_in), Do=diag(dinv_out)
    The gathered table is g_k = Do feat_k, so edge aggregation is an
    unweighted segment-sum; block sums are scaled by cdd = Di*Do to produce
    the next g.  The hop stack stores g_k; the gate recovers
    feat_k = g_k / do per node (valid because no real node has out-degree 0;
    preprocess() checks and falls back to storing feat separately).
  - Everything in the hop loop is bf16 (messages, one-hots, tables, stack):
    PE runs at 1 cycle/row (4x faster than fp32), DVE at 2x, DMA bytes half.
  - Per-core edges sorted by (dst-block of 128, src-quarter), packed into
    128-edge tiles.  The full node table lives in HBM; source rows are
    fetched with batched dma_gather (int16 indices, 4 table quarters).
    A batched DVE is_equal builds 0/1 one-hots [128e x 128d]; PE matmuls
    (lhsT=onehot, rhs=msg) accumulate block sums [128d x 64f] directly in
    PSUM (no transpose needed).
  - MLP (512->64 relu ->64 relu) on PE in bf16; adaptive gate on DVE/ACT.
"""

import os
import sys

for p in ("/opt/trn_rl_repo", "/root/.axon_site/_ro/trn_rl_repo"):
    if os.path.isdir(p) and p not in sys.path:
        sys.path.insert(0, p)

import numpy as np
import ml_dtypes

import concourse.bass as bass
import concourse.bacc as bacc
import concourse.tile as tile
from concourse import mybir
from concourse.bass import AP
from concourse.bass_utils import run_bass_kernel_spmd
from concourse.masks import make_identity
from concourse import library_config

# ----------------------------------------------------------------------------
# problem constants (hardcoded per contract)
N_NODES = 100000
N_EDGES = 3200000
IN_F, HID, OUT_F = 512, 64, 64
K_HOPS = 10
N_CORES = 8

NP_PAD = 100352            # 784 * 128
PER = NP_PAD // N_CORES    # 12544 = 98 * 128
BLOCKS = PER // 128        # 98
F = OUT_F                  # 64
NQ = 4                     # table quarters (int16 index range)
CH = 4                     # dst-blocks per gather chunk

_f32 = mybir.dt.float32
_bf16 = mybir.dt.bfloat16
_i16 = mybir.dt.int16
_np_bf16 = ml_dtypes.bfloat16


# ----------------------------------------------------------------------------
# This walrus build supports at most ONE sync wait per instruction.  After
# Tile has scheduled and assigned semaphores, split any instruction carrying
# N>1 waits into (N-1) same-engine NOPs (engine streams are FIFO, so waits on
# preceding nops gate the instruction identically) + the instruction with 1.
_WSPLIT_CTR = [0]


def split_excess_waits(nc):
    n_split = 0
    for bb in nc.main_func.blocks:
        il = bb.instructions  # live list
        i = 0
        while i < len(il):
            inst = il[i]
            si = inst.sync_info
            if si is None:
                i += 1
                continue
            waits = list(si.on_wait)
            if len(waits) <= 1:
                i += 1
                continue
            for w in waits[:-1]:
                _WSPLIT_CTR[0] += 1
                nop = mybir.InstNoOp(
                    name=f"WSPLIT-{_WSPLIT_CTR[0]}", ins=[], outs=[]
                )
                nop.engine = inst.engine
                nop.sync_info = mybir.SyncInfo(on_wait=[w], on_update=[])
                nc.register_instruction(nop, overwrite=True)
                il.insert(i, nop)
                i += 1
            inst.sync_info = mybir.SyncInfo(
                on_wait=[waits[-1]], on_update=list(si.on_update)
            )
            n_split += 1
            i += 1
    return n_split


# ----------------------------------------------------------------------------
def preprocess(edge_index: np.ndarray):
    """Static per-core edge tiling for the (block, quarter)-run layout.

    Slot stream order (identical across cores): for chunk -> for quarter ->
    for block in chunk -> k_bq*128 slots (padded with idx=0 / dstl=-1).
    k_bq is the max over cores of ceil(count_bq/128), so the compiled
    program structure is shared by all cores.
    """
    QTR = NP_PAD // NQ
    src = np.ascontiguousarray(edge_index[0]).astype(np.int64)
    dst = np.ascontiguousarray(edge_index[1]).astype(np.int64)

    deg_out = np.bincount(src, minlength=N_NODES).astype(np.float64)
    deg_in = np.bincount(dst, minlength=N_NODES).astype(np.float64)
    dinv_out = np.where(
        deg_out > 0, 1.0 / np.sqrt(np.maximum(deg_out, 1.0)), 0.0
    ).astype(np.float32)
    dinv_in = np.where(
        deg_in > 0, 1.0 / np.sqrt(np.maximum(deg_in, 1.0)), 0.0
    ).astype(np.float32)
    fold_do = bool((deg_out > 0).all())
    r_out = np.where(dinv_out > 0, 1.0 / np.maximum(dinv_out, 1e-30), 0.0)
    dinv_out_p = np.zeros(NP_PAD, np.float32)
    dinv_out_p[:N_NODES] = dinv_out
    dinv_in_p = np.zeros(NP_PAD, np.float32)
    dinv_in_p[:N_NODES] = dinv_in
    r_out_p = np.zeros(NP_PAD, np.float32)
    r_out_p[:N_NODES] = r_out

    # per-core edge lists grouped by (block, quarter), dst-sorted inside
    core_of = dst // PER
    per_core = []
    counts = np.zeros((N_CORES, BLOCKS, NQ), np.int64)
    for c in range(N_CORES):
        sel = np.nonzero(core_of == c)[0]
        dl = (dst[sel] - c * PER).astype(np.int64)
        sc = src[sel]
        key = (dl // 128) * NQ + sc // QTR
        order = np.argsort(key, kind="stable")
        dl, sc, key = dl[order], sc[order], key[order]
        per_core.append((sc, dl, key))
        cnt = np.bincount(key, minlength=BLOCKS * NQ)
        counts[c] = cnt.reshape(BLOCKS, NQ)

    k_bq = (counts.max(axis=0) + 127) // 128  # [BLOCKS, NQ] shared
    # ensure every block has at least one tile (so its PSUM gets written)
    empty = k_bq.sum(axis=1) == 0
    k_bq[empty, 0] = 1

    chunks = [
        list(range(c0, min(c0 + CH, BLOCKS))) for c0 in range(0, BLOCKS, CH)
    ]

    # segment offsets in slot units, in stream order
    seg_off = []  # [chunk][q] -> slot offset
    off = 0
    for blist in chunks:
        row = []
        for q in range(NQ):
            row.append(off)
            off += 128 * int(sum(k_bq[b][q] for b in blist))
        seg_off.append(row)
    S = off                      # total slots
    T_total = S // 128           # total tiles / dstl columns

    # stream position of each (b, q) run
    run_off = {}  # (b, q) -> slot offset
    for ci, blist in enumerate(chunks):
        for q in range(NQ):
            o = seg_off[ci][q]
            for b in blist:
                run_off[(b, q)] = o
                o += 128 * int(k_bq[b][q])

    idx16_all = np.zeros((N_CORES, 128, S // 16), np.int16)
    dstl_all = np.full((N_CORES, 128, T_total), -1.0, _np_bf16)
    for c in range(N_CORES):
        sc, dl, key = per_core[c]
        starts = np.concatenate([[0], np.cumsum(counts[c].ravel())])
        idx_flat = np.zeros(S, np.int16)
        dstl_flat = np.full(S, -1.0, np.float32)
        for b in range(BLOCKS):
            for q in range(NQ):
                lo, hi = starts[b * NQ + q], starts[b * NQ + q + 1]
                n = hi - lo
                if n == 0:
                    continue
                base = run_off[(b, q)]
                idx_flat[base : base + n] = (sc[lo:hi] - q * QTR).astype(
                    np.int16
                )
                dstl_flat[base : base + n] = (dl[lo:hi] - b * 128).astype(
                    np.float32
                )
        # idx16 wrapped: index i -> partition i%16, col i//16; replicated x8
        w = idx_flat.reshape(S // 16, 16).T  # [16, S//16]
        idx16_all[c] = np.tile(w, (8, 1))
        dstl_all[c] = dstl_flat.reshape(T_total, 128).T.astype(_np_bf16)

    def shard_table(vec):
        out = np.zeros((N_CORES, 128, BLOCKS), np.float32)
        for c in range(N_CORES):
            out[c] = vec[c * PER : (c + 1) * PER].reshape(BLOCKS, 128).T
        return out

    return {
        "idx16_all": idx16_all,
        "dstl_all": dstl_all,
        "k_bq": k_bq,
        "chunks": chunks,
        "seg_off": seg_off,
        "run_off": run_off,
        "S": S,
        "T_total": T_total,
        "fold_do": fold_do,
        "do_sh": shard_table(dinv_out_p),
        "di_sh": shard_table(dinv_in_p),
        "r_sh": shard_table(r_out_p),
        "cdd_sh": shard_table(dinv_in_p * dinv_out_p),
    }


# ----------------------------------------------------------------------------
def build_kernel(pp, n_hops=None, skip_gather=False, skip_scatter=False,
                 skip_allgather=False, single_packet=False, skip_mlp=False,
                 skip_gate=False, n_queues=4, shared_ag=True,
                 skip_lib=False):
    if n_hops is None:
        n_hops = K_HOPS
    QTR = NP_PAD // NQ
    k_bq = pp["k_bq"]
    chunks = pp["chunks"]
    seg_off = pp["seg_off"]
    run_off = pp["run_off"]
    S = pp["S"]
    T_total = pp["T_total"]
    fold_do = pp["fold_do"]
    KH = n_hops + 1

    nc = bacc.Bacc(None, num_swdge_queues=4)

    x_sh = nc.dram_tensor("x_sh", [PER, IN_F], _bf16, kind="ExternalInput")
    w1 = nc.dram_tensor("w1", [IN_F, HID], _bf16, kind="ExternalInput")
    w2 = nc.dram_tensor("w2", [HID, OUT_F], _bf16, kind="ExternalInput")
    s_rep = nc.dram_tensor("s_rep", [128, F], _f32, kind="ExternalInput")
    do_t = nc.dram_tensor("do_sh", [128, BLOCKS], _f32, kind="ExternalInput")
    di_t = nc.dram_tensor("di_sh", [128, BLOCKS], _f32, kind="ExternalInput")
    r_t = nc.dram_tensor("r_sh", [128, BLOCKS], _f32, kind="ExternalInput")
    cdd_t = nc.dram_tensor("cdd_sh", [128, BLOCKS], _f32, kind="ExternalInput")
    idx16_t = nc.dram_tensor(
        "idx16_t", [128, S // 16], _i16, kind="ExternalInput"
    )
    dstl_t = nc.dram_tensor(
        "dstl_t", [128, T_total], _bf16, kind="ExternalInput"
    )
    out_sh = nc.dram_tensor("out_sh", [PER, F], _f32, kind="ExternalOutput")

    # hop stack; hstK[k] doubles as this core's AllGather input shard.
    # With fold_do it stores g_k = cdd*sum (and g_0 = do*h); otherwise it
    # stores feat_k and a separate gsh holds the table shard.
    hstK = nc.dram_tensor("hstK", [KH, PER, F], _f32)
    gsh = None if fold_do else nc.dram_tensor("gsh", [PER, F], _f32)
    _ag_space = "Shared" if shared_ag else "Local"
    gfa = nc.dram_tensor("gfa", [NP_PAD, F], _f32, addr_space=_ag_space)
    gfb = nc.dram_tensor("gfb", [NP_PAD, F], _f32, addr_space=_ag_space)

    groups = [list(range(N_CORES))]

    with tile.TileContext(nc) as tc:
        with (
            tc.tile_pool(name="const", bufs=1) as constp,
            tc.tile_pool(name="mlp", bufs=3) as mlpp,
            tc.tile_pool(name="psum", bufs=2, space="PSUM") as psp,
            tc.tile_pool(name="msg", bufs=8) as msgp,
            tc.tile_pool(name="idxp", bufs=8) as idxp,
            tc.tile_pool(name="oh", bufs=8) as ohp,
            tc.tile_pool(name="small", bufs=8) as smallp,
        ):
            # ---- constants in SBUF
            ident = constp.tile([128, 128], _bf16)
            make_identity(nc, ident[:])
            iota = constp.tile([128, 128], _bf16)
            nc.gpsimd.iota(
                iota[:], pattern=[[1, 128]], base=0, channel_multiplier=0,
                allow_small_or_imprecise_dtypes=True,
            )
            w1sb = constp.tile([128, 4 * HID], _bf16)
            for k in range(4):
                nc.sync.dma_start(
                    w1sb[:, k * HID : (k + 1) * HID],
                    w1[k * 128 : (k + 1) * 128, :],
                )
            w2sb = constp.tile([HID, OUT_F], _bf16)
            nc.sync.dma_start(w2sb[:], w2[:])
            srepsb = constp.tile([128, F], _f32)
            nc.sync.dma_start(srepsb[:], s_rep[:])
            dosb = constp.tile([128, BLOCKS], _f32)
            nc.sync.dma_start(dosb[:], do_t[:])
            scsb = constp.tile([128, BLOCKS], _f32)  # per-hop output scale
            nc.sync.dma_start(scsb[:], cdd_t[:] if fold_do else di_t[:])
            gatesb = constp.tile([128, BLOCKS], _f32)  # gate recovery scale
            nc.sync.dma_start(gatesb[:], r_t[:])
            cddsb = None
            if not fold_do:
                cddsb = constp.tile([128, BLOCKS], _f32)
                nc.sync.dma_start(cddsb[:], cdd_t[:])
            dstlsb = constp.tile([128, T_total], _bf16)
            nc.sync.dma_start(dstlsb[:], dstl_t[:])

            # switch the Q7 library to 'mlp' (dma_gather lives there); the
            # critical section orders it after iota/identity (standard lib)
            # and before all gathers.
            if not skip_lib:
                with tc.tile_critical():
                    nc.gpsimd.load_library(library_config.mlp)

            # ---- Phase A: MLP over this core's shard
            for b in range(BLOCKS) if not skip_mlp else []:
                xb = mlpp.tile([128, IN_F], _bf16, tag="xb")
                nc.sync.dma_start(xb[:], x_sh[b * 128 : (b + 1) * 128, :])
                xT = mlpp.tile([128, IN_F], _bf16, tag="xT")
                for k in range(4):
                    pst = psp.tile([128, 128], _bf16, tag="t128")
                    nc.tensor.transpose(
                        pst[:], xb[:, k * 128 : (k + 1) * 128], ident[:]
                    )
                    nc.vector.tensor_copy(xT[:, k * 128 : (k + 1) * 128], pst[:])
                ph1 = psp.tile([HID, 128], _f32, tag="p64x128")
                for k in range(4):
                    nc.tensor.matmul(
                        ph1[:],
                        lhsT=w1sb[:, k * HID : (k + 1) * HID],
                        rhs=xT[:, k * 128 : (k + 1) * 128],
                        start=(k == 0),
                        stop=(k == 3),
                    )
                h1T = mlpp.tile([HID, 128], _bf16, tag="h1T")
                nc.scalar.activation(
                    h1T[:], ph1[:], mybir.ActivationFunctionType.Relu
                )
                ph2 = psp.tile([128, OUT_F], _f32, tag="p128x64")
                nc.tensor.matmul(
                    ph2[:], lhsT=h1T[:], rhs=w2sb[:], start=True, stop=True
                )
                # relu then scale by do (fold) / store h and g separately
                hb = mlpp.tile([128, F], _f32, tag="hb")
                nc.scalar.activation(
                    hb[:], ph2[:], mybir.ActivationFunctionType.Relu
                )
                if fold_do:
                    gb = mlpp.tile([128, F], _f32, tag="gb")
                    nc.vector.tensor_scalar_mul(gb[:], hb[:], dosb[:, b : b + 1])
                    nc.sync.dma_start(hstK[0, b * 128 : (b + 1) * 128, :], gb[:])
                else:
                    nc.sync.dma_start(hstK[0, b * 128 : (b + 1) * 128, :], hb[:])
                    gb = mlpp.tile([128, F], _f32, tag="gb")
                    nc.vector.tensor_scalar_mul(gb[:], hb[:], dosb[:, b : b + 1])
                    nc.sync.dma_start(gsh[b * 128 : (b + 1) * 128, :], gb[:])

            if not (skip_allgather or skip_mlp):
                ag_in = hstK[0] if fold_do else gsh[:]
                nc.gpsimd.collective_compute(
                    "AllGather", mybir.AluOpType.bypass, replica_groups=groups,
                    ins=[ag_in], outs=[gfa[:]],
                )

            # ---- Phase B: propagation hops
            for k in range(1, n_hops + 1):
                gin = gfa if (k % 2 == 1) else gfb
                gout = gfb if (k % 2 == 1) else gfa
                for ci, blist in enumerate(chunks):
                    msgs = [None] * NQ
                    for q in range(NQ):
                        nb = int(sum(k_bq[b][q] for b in blist))
                        if nb == 0:
                            continue
                        n = nb * 128
                        off = seg_off[ci][q]
                        idxsl = idxp.tile([128, n // 16], _i16, tag="idxsl")
                        nc.sync.dma_start(
                            idxsl[:],
                            idx16_t[:, off // 16 : (off + n) // 16],
                        )
                        msg = msgp.tile([128, nb * F], _f32, tag="msg")
                        m = msg[:]
                        out3 = AP(m.tensor, m.offset, [m.ap[0], [F, nb], [1, F]])
                        if not skip_gather:
                            nc.gpsimd.dma_gather(
                                out_ap=out3,
                                in_ap=gin[q * QTR : (q + 1) * QTR, :],
                                idxs_ap=idxsl[:],
                                num_idxs=n,
                                num_idxs_reg=n,
                                elem_size=F,
                                single_packet=single_packet,
                                queue_num=q % n_queues,
                            )
                        # convert once per msg group for the bf16 matmuls
                        msgb = msgp.tile([128, nb * F], _bf16, tag="msgb")
                        nc.vector.tensor_copy(msgb[:], msg[:])
                        msgs[q] = msgb
                    for b in blist:
                        if skip_scatter:
                            hk = smallp.tile([128, F], _f32, tag="hk")
                            nc.vector.memset(hk[:], 0.0)
                            nc.sync.dma_start(
                                hstK[k, b * 128 : (b + 1) * 128, :], hk[:]
                            )
                            if not fold_do:
                                nc.sync.dma_start(
                                    gsh[b * 128 : (b + 1) * 128, :], hk[:]
                                )
                            continue
                        # accumulate block sum ps[d, f] directly
                        ps = psp.tile([128, F], _f32, tag="p128x64")
                        total = int(k_bq[b].sum())
                        cnt_i = 0
                        for q in range(NQ):
                            kk = int(k_bq[b][q])
                            if kk == 0:
                                continue
                            tcol0 = run_off[(b, q)] // 128
                            oh = ohp.tile([128, kk * 128], _bf16, tag="oh")
                            a = dstlsb[:, tcol0 : tcol0 + kk]
                            in0 = AP(
                                a.tensor, a.offset,
                                [a.ap[0], a.ap[1], [0, 128]],
                            )
                            i_ = iota[:]
                            in1 = AP(
                                i_.tensor, i_.offset,
                                [i_.ap[0], [0, kk], i_.ap[1]],
                            )
                            o_ = oh[:]
                            outap = AP(
                                o_.tensor, o_.offset,
                                [o_.ap[0], [128, kk], [1, 128]],
                            )
                            nc.vector.tensor_tensor(
                                out=outap, in0=in0, in1=in1,
                                op=mybir.AluOpType.is_equal,
                            )
                            mcol0 = (run_off[(b, q)] - seg_off[ci][q]) // 128
                            for j in range(kk):
                                nc.tensor.matmul(
                                    ps[:],
                                    lhsT=oh[:, j * 128 : (j + 1) * 128],
                                    rhs=msgs[q][
                                        :, (mcol0 + j) * F : (mcol0 + j + 1) * F
                                    ],
                                    start=(cnt_i == 0),
                                    stop=(cnt_i == total - 1),
                                )
                                cnt_i += 1
                        # scale into the next table/stack entry
                        gk = smallp.tile([128, F], _f32, tag="gk")
                        nc.vector.tensor_scalar_mul(
                            gk[:], ps[:], scsb[:, b : b + 1]
                        )
                        nc.sync.dma_start(
                            hstK[k, b * 128 : (b + 1) * 128, :], gk[:]
                        )
                        if not fold_do:
                            g2 = smallp.tile([128, F], _f32, tag="g2")
                            nc.vector.tensor_scalar_mul(
                                g2[:], ps[:], cddsb[:, b : b + 1]
                            )
                            nc.sync.dma_start(
                                gsh[b * 128 : (b + 1) * 128, :], g2[:]
                            )
                if k < n_hops and not skip_allgather:
                    ag_in = hstK[k] if fold_do else gsh[:]
                    nc.gpsimd.collective_compute(
                        "AllGather", mybir.AluOpType.bypass,
                        replica_groups=groups, ins=[ag_in], outs=[gout[:]],
                    )

            # ---- Phase C: adaptive gate
            if skip_gate:
                for b in range(BLOCKS):
                    ob = smallp.tile([128, F], _f32, tag="ob")
                    nc.vector.memset(ob[:], 0.0)
                    nc.sync.dma_start(out_sh[b * 128 : (b + 1) * 128, :], ob[:])
            for b in range(BLOCKS) if not skip_gate else []:
                hbt = mlpp.tile([128, KH * F], _f32, tag="hbt")
                hsrc = hstK[0]  # [PER, F] slab; hops strided by PER*F
                hap = AP(
                    hsrc.tensor, hsrc.offset + b * 128 * F,
                    [[F, 128], [PER * F, KH], [1, F]],
                )
                nc.sync.dma_start(hbt[:], hap)
                if fold_do:
                    # recover feat_k = g_k / do  (r = 1/do per node)
                    hh = mlpp.tile([128, KH * F], _f32, tag="hh")
                    nc.vector.tensor_scalar_mul(
                        hh[:], hbt[:], gatesb[:, b : b + 1]
                    )
                else:
                    hh = hbt
                tmp = mlpp.tile([128, KH * F], _f32, tag="tmp")
                hb3 = AP(hh[:].tensor, hh[:].offset, [hh[:].ap[0], [F, KH], [1, F]])
                sr = srepsb[:]
                sr3 = AP(sr.tensor, sr.offset, [sr.ap[0], [0, KH], [1, F]])
                t3 = AP(tmp[:].tensor, tmp[:].offset, [tmp[:].ap[0], [F, KH], [1, F]])
                nc.vector.tensor_tensor(
                    out=t3, in0=hb3, in1=sr3, op=mybir.AluOpType.mult
                )
                sc = smallp.tile([128, KH], _f32, tag="sc")
                nc.vector.tensor_reduce(
                    sc[:], t3, axis=mybir.AxisListType.X, op=mybir.AluOpType.add
                )
                scs = smallp.tile([128, KH], _f32, tag="scs")
                nc.scalar.activation(
                    scs[:], sc[:], mybir.ActivationFunctionType.Sigmoid
                )
                tmp2 = mlpp.tile([128, KH * F], _f32, tag="tmp2")
                s3 = AP(scs[:].tensor, scs[:].offset, [scs[:].ap[0], [1, KH], [0, F]])
                t23 = AP(tmp2[:].tensor, tmp2[:].offset, [tmp2[:].ap[0], [F, KH], [1, F]])
                nc.vector.tensor_tensor(
                    out=t23, in0=hb3, in1=s3, op=mybir.AluOpType.mult
                )
                ob = smallp.tile([128, F], _f32, tag="ob")
                t2r = AP(tmp2[:].tensor, tmp2[:].offset, [tmp2[:].ap[0], [1, F], [F, KH]])
                nc.vector.tensor_reduce(
                    ob[:], t2r, axis=mybir.AxisListType.X, op=mybir.AluOpType.add
                )
                nc.sync.dma_start(out_sh[b * 128 : (b + 1) * 128, :], ob[:])

    nc.compile()
    split_excess_waits(nc)
    return nc


# ----------------------------------------------------------------------------
def make_in_maps(pp, x, W1, W2, s):
    x_pad = np.zeros((NP_PAD, IN_F), _np_bf16)
    x_pad[:N_NODES] = x.astype(_np_bf16)
    s_rep = np.repeat(
        np.asarray(s, np.float32).reshape(1, F), 128, axis=0
    ).astype(np.float32)
    in_maps = []
    for c in range(N_CORES):
        in_maps.append(
            {
                "x_sh": np.ascontiguousarray(x_pad[c * PER : (c + 1) * PER]),
                "w1": np.asarray(W1, np.float32).astype(_np_bf16),
                "w2": np.asarray(W2, np.float32).astype(_np_bf16),
                "s_rep": s_rep,
                "do_sh": np.ascontiguousarray(pp["do_sh"][c]),
                "di_sh": np.ascontiguousarray(pp["di_sh"][c]),
                "r_sh": np.ascontiguousarray(pp["r_sh"][c]),
                "cdd_sh": np.ascontiguousarray(pp["cdd_sh"][c]),
                "idx16_t": np.ascontiguousarray(pp["idx16_all"][c]),
                "dstl_t": np.ascontiguousarray(pp["dstl_all"][c]),
            }
        )
    return in_maps


KERNEL_KWARGS = {}


def kernel(x, edge_index, W1, W2, s) -> np.ndarray:
    x = np.asarray(x, np.float32)
    edge_index = np.asarray(edge_index)
    W1 = np.asarray(W1, np.float32)
    W2 = np.asarray(W2, np.float32)
    s = np.asarray(s, np.float32)

    pp = preprocess(edge_index)
    nc = build_kernel(pp, **KERNEL_KWARGS)
    in_maps = make_in_maps(pp, x, W1, W2, s)

    res = run_bass_kernel_spmd(nc, in_maps, list(range(N_CORES)))
    out = np.concatenate(
        [res.results[c]["out_sh"] for c in range(N_CORES)], axis=0
    )
    return np.ascontiguousarray(out[:N_NODES])


if __name__ == "__main__":
    rng = np.random.default_rng(0)
    x = rng.standard_normal((N_NODES, IN_F), dtype=np.float32)
    ei = rng.integers(0, N_NODES, (2, N_EDGES)).astype(np.int64)
    W1 = rng.standard_normal((IN_F, HID), dtype=np.float32) / np.sqrt(IN_F)
    W2 = rng.standard_normal((HID, OUT_F), dtype=np.float32) / np.sqrt(HID)
    s = rng.standard_normal((OUT_F, 1), dtype=np.float32) / np.sqrt(OUT_F)
    out = kernel(x=x, edge_index=ei, W1=W1, W2=W2, s=s)
    print("out", out.shape, out.dtype, float(np.abs(out).mean()))


# revision 60
# speedup vs baseline: 1.0351x; 1.0351x over previous
"""DAGNN (gnn_message_passing) Trainium2 Bass kernel, 8-core SPMD.

Strategy:
  - Nodes padded 100000 -> 100352 (784*128); 8 dst-shards of 12544 rows.
  - Symmetric norm factored into per-node scales:
        feat_{k+1} = Di A Do feat_k,  Di=diag(dinv_in), Do=diag(dinv_out)
    The gathered table is g_k = Do feat_k, so edge aggregation is an
    unweighted segment-sum; block sums are scaled by cdd = Di*Do to produce
    the next g.  The hop stack stores g_k; the gate recovers
    feat_k = g_k / do per node (valid because no real node has out-degree 0;
    preprocess() checks and falls back to storing feat separately).
  - Everything in the hop loop is bf16 (messages, one-hots, tables, stack):
    PE runs at 1 cycle/row (4x faster than fp32), DVE at 2x, DMA bytes half.
  - Per-core edges sorted by (dst-block of 128, src-quarter), packed into
    128-edge tiles.  The full node table lives in HBM; source rows are
    fetched with batched dma_gather (int16 indices, 4 table quarters).
    A batched DVE is_equal builds 0/1 one-hots [128e x 128d]; PE matmuls
    (lhsT=onehot, rhs=msg) accumulate block sums [128d x 64f] directly in
    PSUM (no transpose needed).
  - MLP (512->64 relu ->64 relu) on PE in bf16; adaptive gate on DVE/ACT.
"""

import os
import sys

for p in ("/opt/trn_rl_repo", "/root/.axon_site/_ro/trn_rl_repo"):
    if os.path.isdir(p) and p not in sys.path:
        sys.path.insert(0, p)

import numpy as np
import ml_dtypes

import concourse.bass as bass
import concourse.bacc as bacc
import concourse.tile as tile
from concourse import mybir
from concourse.bass import AP
from concourse.bass_utils import run_bass_kernel_spmd
from concourse.masks import make_identity
from concourse import library_config

# ----------------------------------------------------------------------------
# problem constants (hardcoded per contract)
N_NODES = 100000
N_EDGES = 3200000
IN_F, HID, OUT_F = 512, 64, 64
K_HOPS = 10
N_CORES = 8

NP_PAD = 100352            # 784 * 128
PER = NP_PAD // N_CORES    # 12544 = 98 * 128
BLOCKS = PER // 128        # 98
F = OUT_F                  # 64
NQ = 4                     # table quarters (int16 index range)
CH = 4                     # dst-blocks per gather chunk

_f32 = mybir.dt.float32
_bf16 = mybir.dt.bfloat16
_i16 = mybir.dt.int16
_np_bf16 = ml_dtypes.bfloat16


# ----------------------------------------------------------------------------
# This walrus build supports at most ONE sync wait per instruction.  After
# Tile has scheduled and assigned semaphores, split any instruction carrying
# N>1 waits into (N-1) same-engine NOPs (engine streams are FIFO, so waits on
# preceding nops gate the instruction identically) + the instruction with 1.
_WSPLIT_CTR = [0]


def split_excess_waits(nc):
    n_split = 0
    for bb in nc.main_func.blocks:
        il = bb.instructions  # live list
        i = 0
        while i < len(il):
            inst = il[i]
            si = inst.sync_info
            if si is None:
                i += 1
                continue
            waits = list(si.on_wait)
            if len(waits) <= 1:
                i += 1
                continue
            for w in waits[:-1]:
                _WSPLIT_CTR[0] += 1
                nop = mybir.InstNoOp(
                    name=f"WSPLIT-{_WSPLIT_CTR[0]}", ins=[], outs=[]
                )
                nop.engine = inst.engine
                nop.sync_info = mybir.SyncInfo(on_wait=[w], on_update=[])
                nc.register_instruction(nop, overwrite=True)
                il.insert(i, nop)
                i += 1
            inst.sync_info = mybir.SyncInfo(
                on_wait=[waits[-1]], on_update=list(si.on_update)
            )
            n_split += 1
            i += 1
    return n_split


# ----------------------------------------------------------------------------
def preprocess(edge_index: np.ndarray):
    """Static per-core edge tiling for the (block, quarter)-run layout.

    Slot stream order (identical across cores): for chunk -> for quarter ->
    for block in chunk -> k_bq*128 slots (padded with idx=0 / dstl=-1).
    k_bq is the max over cores of ceil(count_bq/128), so the compiled
    program structure is shared by all cores.
    """
    QTR = NP_PAD // NQ
    src = np.ascontiguousarray(edge_index[0]).astype(np.int64)
    dst = np.ascontiguousarray(edge_index[1]).astype(np.int64)

    deg_out = np.bincount(src, minlength=N_NODES).astype(np.float64)
    deg_in = np.bincount(dst, minlength=N_NODES).astype(np.float64)
    dinv_out = np.where(
        deg_out > 0, 1.0 / np.sqrt(np.maximum(deg_out, 1.0)), 0.0
    ).astype(np.float32)
    dinv_in = np.where(
        deg_in > 0, 1.0 / np.sqrt(np.maximum(deg_in, 1.0)), 0.0
    ).astype(np.float32)
    fold_do = bool((deg_out > 0).all())
    r_out = np.where(dinv_out > 0, 1.0 / np.maximum(dinv_out, 1e-30), 0.0)
    dinv_out_p = np.zeros(NP_PAD, np.float32)
    dinv_out_p[:N_NODES] = dinv_out
    dinv_in_p = np.zeros(NP_PAD, np.float32)
    dinv_in_p[:N_NODES] = dinv_in
    r_out_p = np.zeros(NP_PAD, np.float32)
    r_out_p[:N_NODES] = r_out

    # per-core edge lists grouped by (block, quarter), dst-sorted inside
    core_of = dst // PER
    per_core = []
    counts = np.zeros((N_CORES, BLOCKS, NQ), np.int64)
    for c in range(N_CORES):
        sel = np.nonzero(core_of == c)[0]
        dl = (dst[sel] - c * PER).astype(np.int64)
        sc = src[sel]
        key = (dl // 128) * NQ + sc // QTR
        order = np.argsort(key, kind="stable")
        dl, sc, key = dl[order], sc[order], key[order]
        per_core.append((sc, dl, key))
        cnt = np.bincount(key, minlength=BLOCKS * NQ)
        counts[c] = cnt.reshape(BLOCKS, NQ)

    k_bq = (counts.max(axis=0) + 127) // 128  # [BLOCKS, NQ] shared
    # ensure every block has at least one tile (so its PSUM gets written)
    empty = k_bq.sum(axis=1) == 0
    k_bq[empty, 0] = 1

    chunks = [
        list(range(c0, min(c0 + CH, BLOCKS))) for c0 in range(0, BLOCKS, CH)
    ]

    # segment offsets in slot units, in stream order
    seg_off = []  # [chunk][q] -> slot offset
    off = 0
    for blist in chunks:
        row = []
        for q in range(NQ):
            row.append(off)
            off += 128 * int(sum(k_bq[b][q] for b in blist))
        seg_off.append(row)
    S = off                      # total slots
    T_total = S // 128           # total tiles / dstl columns

    # stream position of each (b, q) run
    run_off = {}  # (b, q) -> slot offset
    for ci, blist in enumerate(chunks):
        for q in range(NQ):
            o = seg_off[ci][q]
            for b in blist:
                run_off[(b, q)] = o
                o += 128 * int(k_bq[b][q])

    idx16_all = np.zeros((N_CORES, 128, S // 16), np.int16)
    dstl_all = np.full((N_CORES, 128, T_total), -1.0, _np_bf16)
    for c in range(N_CORES):
        sc, dl, key = per_core[c]
        starts = np.concatenate([[0], np.cumsum(counts[c].ravel())])
        idx_flat = np.zeros(S, np.int16)
        dstl_flat = np.full(S, -1.0, np.float32)
        for b in range(BLOCKS):
            for q in range(NQ):
                lo, hi = starts[b * NQ + q], starts[b * NQ + q + 1]
                n = hi - lo
                if n == 0:
                    continue
                base = run_off[(b, q)]
                idx_flat[base : base + n] = (sc[lo:hi] - q * QTR).astype(
                    np.int16
                )
                dstl_flat[base : base + n] = (dl[lo:hi] - b * 128).astype(
                    np.float32
                )
        # idx16 wrapped: index i -> partition i%16, col i//16; replicated x8
        w = idx_flat.reshape(S // 16, 16).T  # [16, S//16]
        idx16_all[c] = np.tile(w, (8, 1))
        dstl_all[c] = dstl_flat.reshape(T_total, 128).T.astype(_np_bf16)

    def shard_table(vec):
        out = np.zeros((N_CORES, 128, BLOCKS), np.float32)
        for c in range(N_CORES):
            out[c] = vec[c * PER : (c + 1) * PER].reshape(BLOCKS, 128).T
        return out

    return {
        "idx16_all": idx16_all,
        "dstl_all": dstl_all,
        "k_bq": k_bq,
        "chunks": chunks,
        "seg_off": seg_off,
        "run_off": run_off,
        "S": S,
        "T_total": T_total,
        "fold_do": fold_do,
        "do_sh": shard_table(dinv_out_p),
        "di_sh": shard_table(dinv_in_p),
        "r_sh": shard_table(r_out_p),
        "cdd_sh": shard_table(dinv_in_p * dinv_out_p),
    }


# ----------------------------------------------------------------------------
def build_kernel(pp, n_hops=None, skip_gather=False, skip_scatter=False,
                 skip_allgather=False, single_packet=False, skip_mlp=False,
                 skip_gate=False, n_queues=4, shared_ag=True,
                 skip_lib=False):
    if n_hops is None:
        n_hops = K_HOPS
    QTR = NP_PAD // NQ
    k_bq = pp["k_bq"]
    chunks = pp["chunks"]
    seg_off = pp["seg_off"]
    run_off = pp["run_off"]
    S = pp["S"]
    T_total = pp["T_total"]
    fold_do = pp["fold_do"]
    KH = n_hops + 1

    nc = bacc.Bacc(None, num_swdge_queues=4)

    x_sh = nc.dram_tensor("x_sh", [PER, IN_F], _bf16, kind="ExternalInput")
    w1 = nc.dram_tensor("w1", [IN_F, HID], _bf16, kind="ExternalInput")
    w2 = nc.dram_tensor("w2", [HID, OUT_F], _bf16, kind="ExternalInput")
    s_rep = nc.dram_tensor("s_rep", [128, F], _f32, kind="ExternalInput")
    do_t = nc.dram_tensor("do_sh", [128, BLOCKS], _f32, kind="ExternalInput")
    di_t = nc.dram_tensor("di_sh", [128, BLOCKS], _f32, kind="ExternalInput")
    r_t = nc.dram_tensor("r_sh", [128, BLOCKS], _f32, kind="ExternalInput")
    cdd_t = nc.dram_tensor("cdd_sh", [128, BLOCKS], _f32, kind="ExternalInput")
    idx16_t = nc.dram_tensor(
        "idx16_t", [128, S // 16], _i16, kind="ExternalInput"
    )
    dstl_t = nc.dram_tensor(
        "dstl_t", [128, T_total], _bf16, kind="ExternalInput"
    )
    out_sh = nc.dram_tensor("out_sh", [PER, F], _f32, kind="ExternalOutput")

    # hop stack; hstK[k] doubles as this core's AllGather input shard.
    # With fold_do it stores g_k = cdd*sum (and g_0 = do*h); otherwise it
    # stores feat_k and a separate gsh holds the table shard.
    hstK = nc.dram_tensor("hstK", [KH, PER, F], _f32)
    gsh = None if fold_do else nc.dram_tensor("gsh", [PER, F], _f32)
    _ag_space = "Shared" if shared_ag else "Local"
    gfa = nc.dram_tensor("gfa", [NP_PAD, F], _f32, addr_space=_ag_space)
    gfb = nc.dram_tensor("gfb", [NP_PAD, F], _f32, addr_space=_ag_space)

    groups = [list(range(N_CORES))]

    with tile.TileContext(nc) as tc:
        with (
            tc.tile_pool(name="const", bufs=1) as constp,
            tc.tile_pool(name="mlp", bufs=3) as mlpp,
            tc.tile_pool(name="psum", bufs=2, space="PSUM") as psp,
            tc.tile_pool(name="msg", bufs=8) as msgp,
            tc.tile_pool(name="idxp", bufs=8) as idxp,
            tc.tile_pool(name="oh", bufs=8) as ohp,
            tc.tile_pool(name="small", bufs=8) as smallp,
        ):
            # ---- constants in SBUF
            ident = constp.tile([128, 128], _bf16)
            make_identity(nc, ident[:])
            iota = constp.tile([128, 128], _bf16)
            nc.gpsimd.iota(
                iota[:], pattern=[[1, 128]], base=0, channel_multiplier=0,
                allow_small_or_imprecise_dtypes=True,
            )
            w1sb = constp.tile([128, 4 * HID], _bf16)
            for k in range(4):
                nc.sync.dma_start(
                    w1sb[:, k * HID : (k + 1) * HID],
                    w1[k * 128 : (k + 1) * 128, :],
                )
            w2sb = constp.tile([HID, OUT_F], _bf16)
            nc.sync.dma_start(w2sb[:], w2[:])
            srepsb = constp.tile([128, F], _f32)
            nc.sync.dma_start(srepsb[:], s_rep[:])
            dosb = constp.tile([128, BLOCKS], _f32)
            nc.sync.dma_start(dosb[:], do_t[:])
            scsb = constp.tile([128, BLOCKS], _f32)  # per-hop output scale
            nc.sync.dma_start(scsb[:], cdd_t[:] if fold_do else di_t[:])
            gatesb = constp.tile([128, BLOCKS], _f32)  # gate recovery scale
            nc.sync.dma_start(gatesb[:], r_t[:])
            cddsb = None
            if not fold_do:
                cddsb = constp.tile([128, BLOCKS], _f32)
                nc.sync.dma_start(cddsb[:], cdd_t[:])
            dstlsb = constp.tile([128, T_total], _bf16)
            nc.sync.dma_start(dstlsb[:], dstl_t[:])

            # switch the Q7 library to 'mlp' (dma_gather lives there); the
            # critical section orders it after iota/identity (standard lib)
            # and before all gathers.
            if not skip_lib:
                with tc.tile_critical():
                    nc.gpsimd.load_library(library_config.mlp)

            # ---- Phase A: MLP over this core's shard
            for b in range(BLOCKS) if not skip_mlp else []:
                xb = mlpp.tile([128, IN_F], _bf16, tag="xb")
                nc.sync.dma_start(xb[:], x_sh[b * 128 : (b + 1) * 128, :])
                xT = mlpp.tile([128, IN_F], _bf16, tag="xT")
                for k in range(4):
                    pst = psp.tile([128, 128], _bf16, tag="t128")
                    nc.tensor.transpose(
                        pst[:], xb[:, k * 128 : (k + 1) * 128], ident[:]
                    )
                    nc.vector.tensor_copy(xT[:, k * 128 : (k + 1) * 128], pst[:])
                ph1 = psp.tile([HID, 128], _f32, tag="p64x128")
                for k in range(4):
                    nc.tensor.matmul(
                        ph1[:],
                        lhsT=w1sb[:, k * HID : (k + 1) * HID],
                        rhs=xT[:, k * 128 : (k + 1) * 128],
                        start=(k == 0),
                        stop=(k == 3),
                    )
                h1T = mlpp.tile([HID, 128], _bf16, tag="h1T")
                nc.scalar.activation(
                    h1T[:], ph1[:], mybir.ActivationFunctionType.Relu
                )
                ph2 = psp.tile([128, OUT_F], _f32, tag="p128x64", bufs=4)
                nc.tensor.matmul(
                    ph2[:], lhsT=h1T[:], rhs=w2sb[:], start=True, stop=True
                )
                # relu then scale by do (fold) / store h and g separately
                hb = mlpp.tile([128, F], _f32, tag="hb")
                nc.scalar.activation(
                    hb[:], ph2[:], mybir.ActivationFunctionType.Relu
                )
                if fold_do:
                    gb = mlpp.tile([128, F], _f32, tag="gb")
                    nc.vector.tensor_scalar_mul(gb[:], hb[:], dosb[:, b : b + 1])
                    nc.scalar.dma_start(hstK[0, b * 128 : (b + 1) * 128, :], gb[:])
                else:
                    nc.scalar.dma_start(hstK[0, b * 128 : (b + 1) * 128, :], hb[:])
                    gb = mlpp.tile([128, F], _f32, tag="gb")
                    nc.vector.tensor_scalar_mul(gb[:], hb[:], dosb[:, b : b + 1])
                    nc.sync.dma_start(gsh[b * 128 : (b + 1) * 128, :], gb[:])

            if not (skip_allgather or skip_mlp):
                ag_in = hstK[0] if fold_do else gsh[:]
                nc.gpsimd.collective_compute(
                    "AllGather", mybir.AluOpType.bypass, replica_groups=groups,
                    ins=[ag_in], outs=[gfa[:]],
                )

            # ---- Phase B: propagation hops
            for k in range(1, n_hops + 1):
                gin = gfa if (k % 2 == 1) else gfb
                gout = gfb if (k % 2 == 1) else gfa
                for ci, blist in enumerate(chunks):
                    msgs = [None] * NQ
                    for q in range(NQ):
                        nb = int(sum(k_bq[b][q] for b in blist))
                        if nb == 0:
                            continue
                        n = nb * 128
                        off = seg_off[ci][q]
                        idxsl = idxp.tile([128, n // 16], _i16, tag="idxsl")
                        nc.sync.dma_start(
                            idxsl[:],
                            idx16_t[:, off // 16 : (off + n) // 16],
                        )
                        msg = msgp.tile([128, nb * F], _f32, tag="msg")
                        m = msg[:]
                        out3 = AP(m.tensor, m.offset, [m.ap[0], [F, nb], [1, F]])
                        if not skip_gather:
                            nc.gpsimd.dma_gather(
                                out_ap=out3,
                                in_ap=gin[q * QTR : (q + 1) * QTR, :],
                                idxs_ap=idxsl[:],
                                num_idxs=n,
                                num_idxs_reg=n,
                                elem_size=F,
                                single_packet=single_packet,
                                queue_num=q % n_queues,
                            )
                        # convert once per msg group for the bf16 matmuls
                        msgb = msgp.tile([128, nb * F], _bf16, tag="msgb")
                        nc.vector.tensor_copy(msgb[:], msg[:])
                        msgs[q] = msgb
                    if skip_scatter:
                        for b in blist:
                            hk = smallp.tile([128, F], _f32, tag="hk")
                            nc.vector.memset(hk[:], 0.0)
                            nc.sync.dma_start(
                                hstK[k, b * 128 : (b + 1) * 128, :], hk[:]
                            )
                            if not fold_do:
                                nc.sync.dma_start(
                                    gsh[b * 128 : (b + 1) * 128, :], hk[:]
                                )
                        continue

                    def build_ohs(b):
                        # one bf16 one-hot tile per (b, q) run via DVE is_equal
                        ohs = [None] * NQ
                        for q in range(NQ):
                            kk = int(k_bq[b][q])
                            if kk == 0:
                                continue
                            tcol0 = run_off[(b, q)] // 128
                            oh = ohp.tile(
                                [128, kk * 128], _bf16, tag="oh", name="oh"
                            )
                            a = dstlsb[:, tcol0 : tcol0 + kk]
                            in0 = AP(
                                a.tensor, a.offset,
                                [a.ap[0], a.ap[1], [0, 128]],
                            )
                            i_ = iota[:]
                            in1 = AP(
                                i_.tensor, i_.offset,
                                [i_.ap[0], [0, kk], i_.ap[1]],
                            )
                            o_ = oh[:]
                            outap = AP(
                                o_.tensor, o_.offset,
                                [o_.ap[0], [128, kk], [1, 128]],
                            )
                            nc.vector.tensor_tensor(
                                out=outap, in0=in0, in1=in1,
                                op=mybir.AluOpType.is_equal,
                            )
                            ohs[q] = oh
                        return ohs

                    # one-hot builds are pipelined one block ahead so the
                    # data-dependent gk scales never head-block the next
                    # block's is_equal in the in-order DVE queue (which would
                    # stall PE waiting for one-hots).
                    cur_ohs = build_ohs(blist[0])
                    for bi, b in enumerate(blist):
                        nxt_ohs = (
                            build_ohs(blist[bi + 1])
                            if bi + 1 < len(blist) else None
                        )
                        # accumulate block sum ps[d, f] directly
                        ps = psp.tile([128, F], _f32, tag="p128x64", bufs=4)
                        total = int(k_bq[b].sum())
                        cnt_i = 0
                        for q in range(NQ):
                            kk = int(k_bq[b][q])
                            if kk == 0:
                                continue
                            oh = cur_ohs[q]
                            mcol0 = (run_off[(b, q)] - seg_off[ci][q]) // 128
                            for j in range(kk):
                                nc.tensor.matmul(
                                    ps[:],
                                    lhsT=oh[:, j * 128 : (j + 1) * 128],
                                    rhs=msgs[q][
                                        :, (mcol0 + j) * F : (mcol0 + j + 1) * F
                                    ],
                                    start=(cnt_i == 0),
                                    stop=(cnt_i == total - 1),
                                )
                                cnt_i += 1
                        # scale into the next table/stack entry; store on the
                        # Activation queue so it can't head-block SP loads
                        gk = smallp.tile([128, F], _f32, tag="gk")
                        nc.vector.tensor_scalar_mul(
                            gk[:], ps[:], scsb[:, b : b + 1]
                        )
                        nc.scalar.dma_start(
                            hstK[k, b * 128 : (b + 1) * 128, :], gk[:]
                        )
                        if not fold_do:
                            g2 = smallp.tile([128, F], _f32, tag="g2")
                            nc.vector.tensor_scalar_mul(
                                g2[:], ps[:], cddsb[:, b : b + 1]
                            )
                            nc.scalar.dma_start(
                                gsh[b * 128 : (b + 1) * 128, :], g2[:]
                            )
                        cur_ohs = nxt_ohs
                if k < n_hops and not skip_allgather:
                    ag_in = hstK[k] if fold_do else gsh[:]
                    nc.gpsimd.collective_compute(
                        "AllGather", mybir.AluOpType.bypass,
                        replica_groups=groups, ins=[ag_in], outs=[gout[:]],
                    )

            # ---- Phase C: adaptive gate
            if skip_gate:
                for b in range(BLOCKS):
                    ob = smallp.tile([128, F], _f32, tag="ob")
                    nc.vector.memset(ob[:], 0.0)
                    nc.sync.dma_start(out_sh[b * 128 : (b + 1) * 128, :], ob[:])
            for b in range(BLOCKS) if not skip_gate else []:
                hbt = mlpp.tile([128, KH * F], _f32, tag="hbt")
                hsrc = hstK[0]  # [PER, F] slab; hops strided by PER*F
                hap = AP(
                    hsrc.tensor, hsrc.offset + b * 128 * F,
                    [[F, 128], [PER * F, KH], [1, F]],
                )
                nc.sync.dma_start(hbt[:], hap)
                if fold_do:
                    # recover feat_k = g_k / do  (r = 1/do per node)
                    hh = mlpp.tile([128, KH * F], _f32, tag="hh")
                    nc.vector.tensor_scalar_mul(
                        hh[:], hbt[:], gatesb[:, b : b + 1]
                    )
                else:
                    hh = hbt
                tmp = mlpp.tile([128, KH * F], _f32, tag="tmp")
                hb3 = AP(hh[:].tensor, hh[:].offset, [hh[:].ap[0], [F, KH], [1, F]])
                sr = srepsb[:]
                sr3 = AP(sr.tensor, sr.offset, [sr.ap[0], [0, KH], [1, F]])
                t3 = AP(tmp[:].tensor, tmp[:].offset, [tmp[:].ap[0], [F, KH], [1, F]])
                nc.vector.tensor_tensor(
                    out=t3, in0=hb3, in1=sr3, op=mybir.AluOpType.mult
                )
                sc = smallp.tile([128, KH], _f32, tag="sc")
                nc.vector.tensor_reduce(
                    sc[:], t3, axis=mybir.AxisListType.X, op=mybir.AluOpType.add
                )
                scs = smallp.tile([128, KH], _f32, tag="scs")
                nc.scalar.activation(
                    scs[:], sc[:], mybir.ActivationFunctionType.Sigmoid
                )
                tmp2 = mlpp.tile([128, KH * F], _f32, tag="tmp2")
                s3 = AP(scs[:].tensor, scs[:].offset, [scs[:].ap[0], [1, KH], [0, F]])
                t23 = AP(tmp2[:].tensor, tmp2[:].offset, [tmp2[:].ap[0], [F, KH], [1, F]])
                nc.vector.tensor_tensor(
                    out=t23, in0=hb3, in1=s3, op=mybir.AluOpType.mult
                )
                ob = smallp.tile([128, F], _f32, tag="ob")
                t2r = AP(tmp2[:].tensor, tmp2[:].offset, [tmp2[:].ap[0], [1, F], [F, KH]])
                nc.vector.tensor_reduce(
                    ob[:], t2r, axis=mybir.AxisListType.X, op=mybir.AluOpType.add
                )
                nc.sync.dma_start(out_sh[b * 128 : (b + 1) * 128, :], ob[:])

    nc.compile()
    split_excess_waits(nc)
    return nc


# ----------------------------------------------------------------------------
def make_in_maps(pp, x, W1, W2, s):
    x_pad = np.zeros((NP_PAD, IN_F), _np_bf16)
    x_pad[:N_NODES] = x.astype(_np_bf16)
    s_rep = np.repeat(
        np.asarray(s, np.float32).reshape(1, F), 128, axis=0
    ).astype(np.float32)
    in_maps = []
    for c in range(N_CORES):
        in_maps.append(
            {
                "x_sh": np.ascontiguousarray(x_pad[c * PER : (c + 1) * PER]),
                "w1": np.asarray(W1, np.float32).astype(_np_bf16),
                "w2": np.asarray(W2, np.float32).astype(_np_bf16),
                "s_rep": s_rep,
                "do_sh": np.ascontiguousarray(pp["do_sh"][c]),
                "di_sh": np.ascontiguousarray(pp["di_sh"][c]),
                "r_sh": np.ascontiguousarray(pp["r_sh"][c]),
                "cdd_sh": np.ascontiguousarray(pp["cdd_sh"][c]),
                "idx16_t": np.ascontiguousarray(pp["idx16_all"][c]),
                "dstl_t": np.ascontiguousarray(pp["dstl_all"][c]),
            }
        )
    return in_maps


KERNEL_KWARGS = {}


def kernel(x, edge_index, W1, W2, s) -> np.ndarray:
    x = np.asarray(x, np.float32)
    edge_index = np.asarray(edge_index)
    W1 = np.asarray(W1, np.float32)
    W2 = np.asarray(W2, np.float32)
    s = np.asarray(s, np.float32)

    pp = preprocess(edge_index)
    nc = build_kernel(pp, **KERNEL_KWARGS)
    in_maps = make_in_maps(pp, x, W1, W2, s)

    res = run_bass_kernel_spmd(nc, in_maps, list(range(N_CORES)))
    out = np.concatenate(
        [res.results[c]["out_sh"] for c in range(N_CORES)], axis=0
    )
    return np.ascontiguousarray(out[:N_NODES])


if __name__ == "__main__":
    rng = np.random.default_rng(0)
    x = rng.standard_normal((N_NODES, IN_F), dtype=np.float32)
    ei = rng.integers(0, N_NODES, (2, N_EDGES)).astype(np.int64)
    W1 = rng.standard_normal((IN_F, HID), dtype=np.float32) / np.sqrt(IN_F)
    W2 = rng.standard_normal((HID, OUT_F), dtype=np.float32) / np.sqrt(HID)
    s = rng.standard_normal((OUT_F, 1), dtype=np.float32) / np.sqrt(OUT_F)
    out = kernel(x=x, edge_index=ei, W1=W1, W2=W2, s=s)
    print("out", out.shape, out.dtype, float(np.abs(out).mean()))
